# revision 9
# baseline (speedup 1.0000x reference)
"""Trainium2 Bass kernel for nn_DPLoss (rank-correlation + pairdist loss).

Computes, on 8 NeuronCores, loss = rank_loss + 0.5 * pairdist_loss where
  rank_loss    = mean(|rank_x - rank_z|) / 32
  pairdist_loss = mean((dist_z - dist_x)^2)
over the 4096x4096 pairwise-distance matrices of x [4096,512] and z [4096,64].

Sharding: core c owns rows [512c, 512c+512) of both distance matrices; each
core reads the full (transposed) x and z to compute its row block, and the
scalar partial sums are combined on the host.

Per 128-row block on each core:
  1. PE computes squared distances via a bf16 Gram matmul plus an fp32
     rank-2 "norms" matmul (nrm_i + nrm_j), ACT does relu+sqrt.
  2. Rank of every entry within its row = position in an ascending sort of
     packed keys  q*4096 + col  (q = 12-bit quantized distance); keys are
     integers < 2^24 held exactly in fp32.  A 78-stage bitonic network of
     DVE tensor_tensor min/max instructions sorts each row; GPSIMD
     local_scatter inverts the permutation to give ranks in column order.
  3. sum|rank_x - rank_z| = sum(max(rx,rz)) - sum(min(rx,rz)) via two fused
     tensor_tensor_reduce passes; pairdist via subtract + square-reduce.
Final partition reduction is a ones-vector matmul.
"""
import numpy as np
import ml_dtypes

import concourse.bass as bass
import concourse.bacc as bacc
import concourse.mybir as mybir
import concourse.tile as tile
from concourse.bass_utils import run_bass_kernel_spmd

F32 = mybir.dt.float32
BF16 = mybir.dt.bfloat16
I32 = mybir.dt.int32
I16 = mybir.dt.int16
OP = mybir.AluOpType
ACTF = mybir.ActivationFunctionType

# problem constants
N = 4096
D_X = 512
D_Z = 64
N_CORES = 8
ROWS_PER_CORE = N // N_CORES
K_DIV = 32.0
LAMBDA_RANK = 1.0
LAMBDA_PAIRDIST = 0.5

NB = 4096                  # distance quantization buckets (12 bits)
WIN_X = (0.0, 48.0)        # validated in numpy_model.py: rel err ~4e-6
WIN_Z = (0.0, 18.0)


def _emit_bitonic_sort(nc, A, B, n):
    """Emit a bitonic ascending row sort of A ([P, n] f32) using B as the
    ping-pong buffer.  Returns the AP holding the sorted rows."""
    dve = nc.vector
    cur, other = A, B
    k = 2
    while k <= n:
        s = k // 2
        while s >= 1:
            if 2 * k <= n:
                VA = cur.rearrange("p (c f) -> p c f", f=2 * k)
                VB = other.rearrange("p (c f) -> p c f", f=2 * k)
                Aa = VA[:, :, 0:k].rearrange("p c (e g) -> p c e g", g=2 * s)
                Ba = VB[:, :, 0:k].rearrange("p c (e g) -> p c e g", g=2 * s)
                Ad = VA[:, :, k:2 * k].rearrange("p c (e g) -> p c e g", g=2 * s)
                Bd = VB[:, :, k:2 * k].rearrange("p c (e g) -> p c e g", g=2 * s)
                dve.tensor_tensor(Ba[:, :, :, 0:s], Aa[:, :, :, 0:s], Aa[:, :, :, s:2 * s], OP.min)
                dve.tensor_tensor(Ba[:, :, :, s:2 * s], Aa[:, :, :, 0:s], Aa[:, :, :, s:2 * s], OP.max)
                dve.tensor_tensor(Bd[:, :, :, 0:s], Ad[:, :, :, 0:s], Ad[:, :, :, s:2 * s], OP.max)
                dve.tensor_tensor(Bd[:, :, :, s:2 * s], Ad[:, :, :, 0:s], Ad[:, :, :, s:2 * s], OP.min)
            else:
                Aa = cur.rearrange("p (e g) -> p e g", g=2 * s)
                Ba = other.rearrange("p (e g) -> p e g", g=2 * s)
                dve.tensor_tensor(Ba[:, :, 0:s], Aa[:, :, 0:s], Aa[:, :, s:2 * s], OP.min)
                dve.tensor_tensor(Ba[:, :, s:2 * s], Aa[:, :, 0:s], Aa[:, :, s:2 * s], OP.max)
            cur, other = other, cur
            s //= 2
        k *= 2
    return cur


def _mk_ap(base, off, dims):
    """Raw AP at element offset `off` with free dims [[step, count], ...]."""
    return bass.AP(base.tensor, base.offset + off,
                   [list(base.ap[0])] + [list(d) for d in dims])


def _emit_bitonic_sort_v2(nc, A, B, n):
    """Bitonic sort with combined ascending+descending instructions (2 DVE
    tensor_tensor ops per stage instead of 4, via 4-free-dim APs)."""
    dve = nc.vector
    cur, other = A, B
    k = 2
    while k <= n:
        s = k // 2
        while s >= 1:
            if 2 * k <= n:
                c = n // (2 * k)
                m = k // (2 * s)
                in_lo = _mk_ap(cur, 0, [[2 * k, c], [k, 2], [2 * s, m], [1, s]])
                in_hi = _mk_ap(cur, s, [[2 * k, c], [k, 2], [2 * s, m], [1, s]])
                out_mn = _mk_ap(other, 0, [[2 * k, c], [k + s, 2], [2 * s, m], [1, s]])
                out_mx = _mk_ap(other, s, [[2 * k, c], [k - s, 2], [2 * s, m], [1, s]])
                dve.tensor_tensor(out_mn, in_lo, in_hi, OP.min)
                dve.tensor_tensor(out_mx, in_lo, in_hi, OP.max)
            else:
                m = n // (2 * s)
                in_lo = _mk_ap(cur, 0, [[2 * s, m], [1, s]])
                in_hi = _mk_ap(cur, s, [[2 * s, m], [1, s]])
                out_mn = _mk_ap(other, 0, [[2 * s, m], [1, s]])
                out_mx = _mk_ap(other, s, [[2 * s, m], [1, s]])
                dve.tensor_tensor(out_mn, in_lo, in_hi, OP.min)
                dve.tensor_tensor(out_mx, in_lo, in_hi, OP.max)
            cur, other = other, cur
            s //= 2
        k *= 2
    return cur


def build_nc(n=N, dx=D_X, dz=D_Z, rows_per_core=ROWS_PER_CORE, n_cores=N_CORES,
             win_x=WIN_X, win_z=WIN_Z, nb=NB, debug_ranks=False, sort_v=1):
    n_blocks = rows_per_core // 128
    assert rows_per_core % 128 == 0
    cc_sz = min(512, n)
    n_cc = n // cc_sz
    fcx_sz = min(dx, 128)
    fcz_sz = min(dz, 128)
    n_fcx = dx // fcx_sz
    n_fcz = dz // fcz_sz

    nc = bacc.Bacc("TRN2", target_bir_lowering=False, debug=False,
                   enable_asserts=True, num_devices=n_cores)

    xT = nc.dram_tensor("xT", [dx, n], BF16, kind="ExternalInput")
    zT = nc.dram_tensor("zT", [dz, n], BF16, kind="ExternalInput")
    xTb = nc.dram_tensor("xTb", [dx, rows_per_core], BF16, kind="ExternalInput")
    zTb = nc.dram_tensor("zTb", [dz, rows_per_core], BF16, kind="ExternalInput")
    out = nc.dram_tensor("out", [1, 2], F32, kind="ExternalOutput")
    if debug_ranks:
        rx_dbg = nc.dram_tensor("rx_dbg", [rows_per_core, n], I16, kind="ExternalOutput")
        rz_dbg = nc.dram_tensor("rz_dbg", [rows_per_core, n], I16, kind="ExternalOutput")
        dx_dbg = nc.dram_tensor("dx_dbg", [rows_per_core, n], F32, kind="ExternalOutput")

    sc_x = float(nb) / (win_x[1] - win_x[0])
    sc_z = float(nb) / (win_z[1] - win_z[0])

    max_ne = 2032
    sc_chunks = []
    base = 0
    while base < n:
        ne = min(max_ne, n - base)
        sc_chunks.append((base, ne))
        base += ne

    with tile.TileContext(nc) as tc:
        with (
            tc.tile_pool(name="persist", bufs=1) as pp,
            tc.tile_pool(name="stream", bufs=3) as sp,
            tc.tile_pool(name="big", bufs=1) as bp,
            tc.tile_pool(name="i16", bufs=1) as ip,
            tc.tile_pool(name="ns", bufs=2) as nsp,
            tc.tile_pool(name="acc", bufs=1) as accp,
            tc.tile_pool(name="psA", bufs=4, space="PSUM") as ppsA,
            tc.tile_pool(name="psB", bufs=2, space="PSUM") as ppsB,
            tc.tile_pool(name="psS", bufs=1, space="PSUM") as ppsS,
        ):
            ones = pp.tile([128, 1], F32, tag="ones")
            nc.gpsimd.memset(ones[:], 1.0)
            pos16 = pp.tile([128, n], I16, tag="pos16")
            nc.gpsimd.iota(pos16[:], pattern=[[1, n]], base=0, channel_multiplier=0)

            rmxb = accp.tile([128, n_blocks], F32, tag="rmxb", name="rmxb")
            rmnb = accp.tile([128, n_blocks], F32, tag="rmnb", name="rmnb")
            pdb = accp.tile([128, n_blocks], F32, tag="pdb", name="pdb")

            # ---- norm rows ----
            # aug_rhs: [2, n]   row0 = ones, row1 = nrm(all points)
            # aug_lhs: [2, rpc] row0 = nrm(this core's rows), row1 = ones
            def build_aug(tag, T_dram, width, n_fc, fc_sz, nrm_row):
                # engines can only address partition ranges starting at 0, so
                # the partition-1 row is filled via SBUF->SBUF DMA
                aug = pp.tile([2, width], F32, tag=f"aug_{tag}", name=f"aug_{tag}")
                nc.vector.memset(aug[0:2, :], 1.0)
                w = min(cc_sz, width)
                for cc in range(width // w):
                    npsum = ppsS.tile([1, w], F32, tag="psS", name=f"np_{tag}{cc}")
                    for fc in range(n_fc):
                        xc = sp.tile([fc_sz, w], BF16, tag="xs", name=f"xc_{tag}{cc}_{fc}")
                        nc.sync.dma_start(xc[:], T_dram.ap()[fc * fc_sz:(fc + 1) * fc_sz,
                                                             cc * w:(cc + 1) * w])
                        sq = sp.tile([fc_sz, w], F32, tag="xsq", name=f"xsq_{tag}{cc}_{fc}")
                        nc.scalar.activation(sq[:], xc[:], ACTF.Square)
                        nc.tensor.matmul(npsum[:], ones[0:fc_sz, :], sq[:],
                                         start=(fc == 0), stop=(fc == n_fc - 1))
                    if nrm_row == 0:
                        nc.scalar.copy(aug[0:1, cc * w:(cc + 1) * w], npsum[:])
                    else:
                        tmp = sp.tile([1, w], F32, tag="rowtmp", name=f"rt_{tag}{cc}")
                        nc.scalar.copy(tmp[:], npsum[:])
                        nc.sync.dma_start(aug[1:2, cc * w:(cc + 1) * w], tmp[:])
                return aug

            aug_rhs_x = build_aug("rx", xT, n, n_fcx, fcx_sz, nrm_row=1)
            aug_rhs_z = build_aug("rz", zT, n, n_fcz, fcz_sz, nrm_row=1)
            aug_lhs_x = build_aug("lx", xTb, rows_per_core, n_fcx, fcx_sz, nrm_row=0)
            aug_lhs_z = build_aug("lz", zTb, rows_per_core, n_fcz, fcz_sz, nrm_row=0)

            xTb_sb = []
            for fc in range(n_fcx):
                t = pp.tile([fcx_sz, rows_per_core], BF16, tag=f"xTb{fc}", name=f"xTb_sb{fc}")
                nc.sync.dma_start(t[:], xTb.ap()[fc * fcx_sz:(fc + 1) * fcx_sz, :])
                xTb_sb.append(t)
            zTb_sb = []
            for fc in range(n_fcz):
                t = pp.tile([fcz_sz, rows_per_core], BF16, tag=f"zTb{fc}", name=f"zTb_sb{fc}")
                nc.sync.dma_start(t[:], zTb.ap()[fc * fcz_sz:(fc + 1) * fcz_sz, :])
                zTb_sb.append(t)

            # ---- main row-block loop ----
            for blk in range(n_blocks):
                r0 = blk * 128

                def build_dist(T_dram, Tb_sb, aug_l, aug_r, n_fc, fc_sz, tag):
                    SQ = bp.tile([128, n], F32, tag=f"big_{tag}", name=f"sq_{tag}{blk}")
                    for cc in range(n_cc):
                        csl = slice(cc * cc_sz, (cc + 1) * cc_sz)
                        bankA = ppsA.tile([128, cc_sz], F32, tag="bankA", name=f"bA{tag}{blk}_{cc}")
                        for fc in range(n_fc):
                            rc = sp.tile([fc_sz, cc_sz], BF16, tag="xs", name=f"rc{tag}{blk}_{cc}_{fc}")
                            nc.sync.dma_start(rc[:], T_dram.ap()[fc * fc_sz:(fc + 1) * fc_sz, csl])
                            nc.tensor.matmul(bankA[:], Tb_sb[fc][:, r0:r0 + 128], rc[:],
                                             start=(fc == 0), stop=(fc == n_fc - 1))
                        bankB = ppsB.tile([128, cc_sz], F32, tag="bankB", name=f"bB{tag}{blk}_{cc}")
                        nc.tensor.matmul(bankB[:], aug_l[0:2, r0:r0 + 128], aug_r[0:2, csl],
                                         start=True, stop=True)
                        ns = nsp.tile([128, cc_sz], F32, tag="ns", name=f"ns{tag}{blk}_{cc}")
                        nc.scalar.copy(ns[:], bankB[:])
                        nc.vector.scalar_tensor_tensor(SQ[:, csl], bankA[:], -2.0, ns[:],
                                                       OP.mult, OP.add)
                    nc.scalar.activation(SQ[:], SQ[:], ACTF.Relu)
                    nc.scalar.activation(SQ[:], SQ[:], ACTF.Sqrt)
                    return SQ

                DX = build_dist(xT, xTb_sb, aug_lhs_x, aug_rhs_x, n_fcx, fcx_sz, "sqx")
                DZ = build_dist(zT, zTb_sb, aug_lhs_z, aug_rhs_z, n_fcz, fcz_sz, "sqz")
                if debug_ranks:
                    nc.sync.dma_start(dx_dbg.ap()[r0:r0 + 128, :], DX[:])

                # pairdist partial
                KX = bp.tile([128, n], F32, tag="big_kx", name=f"kx{blk}")
                KZ = bp.tile([128, n], F32, tag="big_kz", name=f"kz{blk}")
                nc.vector.tensor_tensor(KX[:], DZ[:], DX[:], OP.subtract)
                nc.vector.tensor_tensor(KZ[:], KX[:], KX[:], OP.mult)
                nc.vector.tensor_reduce(pdb[:, blk:blk + 1], KZ[:], mybir.AxisListType.X, OP.add)

                def build_keys(D, KT, lo, sc, tag):
                    nc.vector.tensor_scalar(KT[:], D[:], sc, -lo * sc, OP.mult, OP.add)
                    QI = bp.tile([128, n], I32, tag="big_scratch", name=f"qi{tag}{blk}")
                    nc.vector.tensor_scalar(QI[:], KT[:], 0.0, float(nb - 1), OP.max, OP.min)
                    nc.vector.scalar_tensor_tensor(KT[:], QI[:], 4096.0, pos16[:],
                                                   OP.mult, OP.add)

                build_keys(DX, KX, win_x[0], sc_x, "x")
                build_keys(DZ, KZ, win_z[0], sc_z, "z")
                # DX/DZ dead; their slots recycle for the sort scratch

                def ranks_of(KT, tag):
                    SB = bp.tile([128, n], F32, tag="big_scratch", name=f"sb{tag}{blk}")
                    sorter = _emit_bitonic_sort_v2 if sort_v == 2 else _emit_bitonic_sort
                    sorted_ap = sorter(nc, KT[:], SB[:], n)
                    # j = int(key) & 0xFFF, all in exact-integer domain (the
                    # f32->int cast rounds on HW but truncates in CoreSim; keys
                    # are exact ints so the cast is exact either way)
                    KI = bp.tile([128, n], I32, tag="big_scratch2", name=f"ki{tag}{blk}")
                    nc.vector.tensor_copy(KI[:], sorted_ap)
                    J32 = bp.tile([128, n], I32, tag="big_scratch", name=f"j32{tag}{blk}")
                    nc.vector.tensor_scalar(J32[:], KI[:], 0xFFF, None, OP.bitwise_and)
                    R = ip.tile([128, n], I16, tag=f"r_{tag}", name=f"r{tag}{blk}")
                    T16 = ip.tile([128, n], I16, tag="t16", name=f"t16{tag}{blk}")
                    M16 = ip.tile([128, n], I16, tag="m16", name=f"m16{tag}{blk}")
                    for (cbase, ne) in sc_chunks:
                        last = cbase + ne >= n
                        if cbase == 0 and last:
                            src = ip.tile([128, n], I16, tag="j16", name=f"j16{tag}{blk}")
                            nc.vector.tensor_copy(src[:], J32[:])
                        elif cbase == 0:
                            nc.vector.tensor_scalar(M16[:], J32[:], float(ne), None, OP.is_ge)
                            nc.vector.scalar_tensor_tensor(T16[:], M16[:], -8192.0, J32[:],
                                                           OP.mult, OP.add)
                            src = T16
                        else:
                            nc.vector.tensor_scalar(T16[:], J32[:], float(cbase), None, OP.subtract)
                            if not last:
                                nc.vector.tensor_scalar(M16[:], T16[:], float(ne), None, OP.is_ge)
                                nc.vector.scalar_tensor_tensor(T16[:], M16[:], -8192.0, T16[:],
                                                               OP.mult, OP.add)
                            src = T16
                        nc.gpsimd.local_scatter(R[:, cbase:cbase + ne], pos16[:], src[:],
                                                channels=128, num_elems=ne, num_idxs=n)
                    return R

                RX = ranks_of(KX, "x")
                RZ = ranks_of(KZ, "z")
                if debug_ranks:
                    nc.sync.dma_start(rx_dbg.ap()[r0:r0 + 128, :], RX[:])
                    nc.sync.dma_start(rz_dbg.ap()[r0:r0 + 128, :], RZ[:])

                # sum|rx-rz| = sum(max) - sum(min); f32 intermediates (int16
                # tensor_reduce miscounts on HW)
                TRF = bp.tile([128, n], F32, tag="big_scratch", name=f"trf{blk}")
                nc.vector.tensor_tensor(TRF[:], RX[:], RZ[:], OP.max)
                nc.vector.tensor_reduce(rmxb[:, blk:blk + 1], TRF[:], mybir.AxisListType.X, OP.add)
                TRF2 = bp.tile([128, n], F32, tag="big_scratch2", name=f"trf2{blk}")
                nc.vector.tensor_tensor(TRF2[:], RX[:], RZ[:], OP.min)
                nc.vector.tensor_reduce(rmnb[:, blk:blk + 1], TRF2[:], mybir.AxisListType.X, OP.add)

            res2 = pp.tile([128, 2], F32, tag="res2")
            tmx = accp.tile([128, 1], F32, tag="tmx", name="tmx")
            tmn = accp.tile([128, 1], F32, tag="tmn", name="tmn")
            nc.vector.tensor_reduce(tmx[:], rmxb[:], mybir.AxisListType.X, OP.add)
            nc.vector.tensor_reduce(tmn[:], rmnb[:], mybir.AxisListType.X, OP.add)
            nc.vector.tensor_tensor(res2[:, 0:1], tmx[:], tmn[:], OP.subtract)
            nc.vector.tensor_reduce(res2[:, 1:2], pdb[:], mybir.AxisListType.X, OP.add)
            fin = ppsS.tile([1, 2], F32, tag="psS", name="fin")
            nc.tensor.matmul(fin[:], ones[:], res2[:], start=True, stop=True)
            out_sb = pp.tile([1, 2], F32, tag="out_sb")
            nc.scalar.copy(out_sb[:], fin[:])
            nc.sync.dma_start(out.ap(), out_sb[:])

    nc.compile()
    return nc


_NC_CACHE = {}


def _get_nc():
    if "full" not in _NC_CACHE:
        _NC_CACHE["full"] = build_nc()
    return _NC_CACHE["full"]


def make_in_maps(x, z, n_cores=N_CORES, rows_per_core=ROWS_PER_CORE):
    xT = np.ascontiguousarray(x.T).astype(ml_dtypes.bfloat16)
    zT = np.ascontiguousarray(z.T).astype(ml_dtypes.bfloat16)
    in_maps = []
    for c in range(n_cores):
        rows = slice(c * rows_per_core, (c + 1) * rows_per_core)
        in_maps.append({
            "xT": xT,
            "zT": zT,
            "xTb": np.ascontiguousarray(xT[:, rows]),
            "zTb": np.ascontiguousarray(zT[:, rows]),
        })
    return in_maps


def combine(results, n=N):
    rank_sum = sum(float(r["out"][0, 0]) for r in results)
    pd_sum = sum(float(r["out"][0, 1]) for r in results)
    denom = float(n) * float(n)
    rank_loss = np.float32(rank_sum / denom / K_DIV)
    pairdist = np.float32(pd_sum / denom)
    total = np.float32(LAMBDA_RANK * rank_loss + LAMBDA_PAIRDIST * pairdist)
    return total, rank_loss, pairdist


def kernel(x, z):
    x = np.asarray(x, dtype=np.float32)
    z = np.asarray(z, dtype=np.float32)
    nc = _get_nc()
    res = run_bass_kernel_spmd(nc, make_in_maps(x, z), list(range(N_CORES)))
    return combine(res.results)
